# revision 7
# baseline (speedup 1.0000x reference)
"""Trainium2 Bass kernel for nn_DirectG2PNEFS: 3-layer biLSTM encoder +
autoregressive LSTM decoder with argmax feedback.

Sharding: batch 32 -> 8 cores x 4 (data parallel, weights replicated).

All gate nonlinearities are computed as tanh (one ACT table):
  sigmoid(x) = 0.5*(tanh(x/2)+1)
i/f/o gate rows of every weight table are pre-scaled by 0.5 on the host, so
the device computes t = tanh(raw/2) for them. Cell update (fused STT ops):
  a  = (t_i + 1) * t_g          # = 2*sig_i*tanh_g
  b  = (t_f + 1) * C            # C = 2c state
  C' = 0.5*b + a                # = 2c'
  T  = tanh(0.5*C')             # = tanh(c')
  H' = (t_o + 1) * T            # = 2h
State is carried as 2c / 2h; weight columns consuming an h-input are
pre-scaled by 0.5 on the host.

Gate PSUM layout: [32*g + b, 256] via col-tiled M=4 matmuls
(tile_position (0, 32g)). tanh(i,f) -> PSUM, tanh(g,o) -> SBUF so every
binary DVE op is mixed-space (any partition base) or co-based SBUF.
"""
import os
import numpy as np
from contextlib import ExitStack

import concourse.bass as bass
import concourse.bacc as bacc
import concourse.tile as tile
from concourse import mybir

F32 = mybir.dt.float32
AX = mybir.AxisListType
OP = mybir.AluOpType
ACT = mybir.ActivationFunctionType

H = 256
G4 = 4 * H
VOCAB = 256
B = 32
T = 256
DEC = 128
NCORES = 8
BC = B // NCORES          # batch rows per core

_PROGRAM_CACHE = {}


# ---------------------------------------------------------------- host prep
def _srow():
    s = np.ones(G4, np.float64)
    s[0:H] = 0.5
    s[H:2 * H] = 0.5
    s[3 * H:4 * H] = 0.5
    return s


def _rhs(w):
    """w: (4H, din) -> rhs table (din//128, 128, 4H) fp32."""
    din = w.shape[1]
    return np.ascontiguousarray(w.T).reshape(din // 128, 128, G4).astype(np.float32)


def prep_tables(embedding, enc_params, dec_params, proj_w, proj_b):
    s = _srow()
    E = np.asarray(embedding, np.float64)
    tabs = {}
    for l in range(3):
        for d, dn in ((0, "f"), (1, "b")):
            w_ih, w_hh, b_ih, b_hh = [np.asarray(x, np.float64) for x in enc_params[l][d]]
            if l == 0:
                P = (E @ w_ih.T + b_ih + b_hh) * s[None, :]
                tabs[f"penc0{dn}"] = P.reshape(2, 128, G4).astype(np.float32)
            else:
                tabs[f"wih{l}{dn}"] = _rhs(s[:, None] * w_ih * 0.5)
                tabs[f"bias{l}{dn}"] = (s * (b_ih + b_hh)).astype(np.float32)[None, :]
            if not (l == 2 and d == 0):
                tabs[f"whh{l}{dn}"] = _rhs(s[:, None] * w_hh * 0.5)
    for l in range(3):
        w_ih, w_hh, b_ih, b_hh = [np.asarray(x, np.float64) for x in dec_params[l]]
        tabs[f"wdhh{l}"] = _rhs(s[:, None] * w_hh * 0.5)
        if l == 0:
            wsum = w_ih[:, :H] + w_ih[:, H:]
            P0 = (E @ wsum.T + b_ih + b_hh) * s[None, :]
            tabs["p0"] = P0.reshape(2, 128, G4).astype(np.float32)
            tabs["wx0"] = _rhs(s[:, None] * w_ih * 0.5)
            tabs["bdec0"] = (s * (b_ih + b_hh)).astype(np.float32)[None, :]
        else:
            tabs[f"wdih{l}"] = _rhs(s[:, None] * w_ih * 0.5)
            tabs[f"bdec{l}"] = (s * (b_ih + b_hh)).astype(np.float32)[None, :]
    tabs["projt"] = np.ascontiguousarray(
        (0.5 * np.asarray(proj_w, np.float64)).T).reshape(2, 128, 256).astype(np.float32)
    tabs["projb"] = np.asarray(proj_b, np.float32)[None, :]
    tabs["i128"] = np.eye(128, dtype=np.float32)
    # permutation: rows 0-3 -> cols 0-3, rows 32-35 -> cols 4-7, rest bijective
    perm = np.zeros((64, 64), np.float32)
    rows = list(range(4)) + list(range(32, 36))
    rest = [r for r in range(64) if r not in rows]
    for c, r in enumerate(rows + rest):
        perm[r, c] = 1.0
    tabs["id8"] = perm
    tabs["ones128"] = np.ones((1, 128), np.float32)
    tabs["ones4"] = np.ones((1, 4), np.float32)
    return tabs


def onehot_core(text_slice, t_steps=T):
    oh = np.zeros((VOCAB, t_steps * BC), np.float32)
    for b in range(BC):
        for t in range(t_steps):
            oh[int(text_slice[b, t]), 4 * t + b] = 1.0
    return oh.reshape(2, 128, t_steps * BC)


# ---------------------------------------------------------------- device build
def build_program(t_steps=T, dec_steps=DEC, num_devices=NCORES):
    nc = bacc.Bacc("TRN2", target_bir_lowering=False, debug=False,
                   num_devices=num_devices)
    cb = 1 + t_steps          # col-blocks per hT chunk (block 0 / t_steps = zeros)
    nblk = t_steps // 32

    inp = {}

    def din(name, shape):
        inp[name] = nc.dram_tensor(name, shape, F32, kind="ExternalInput").ap()

    din("onehot", [2, 128, t_steps * BC])
    for dn in "fb":
        din(f"penc0{dn}", [2, 128, G4])
        din(f"whh0{dn}", [2, 128, G4])
        din(f"whh1{dn}", [2, 128, G4])
        din(f"wih1{dn}", [4, 128, G4])
        din(f"wih2{dn}", [4, 128, G4])
        din(f"bias1{dn}", [1, G4])
        din(f"bias2{dn}", [1, G4])
    din("whh2b", [2, 128, G4])
    din("p0", [2, 128, G4])
    din("wx0", [4, 128, G4])
    for l in range(3):
        din(f"wdhh{l}", [2, 128, G4])
    din("wdih1", [2, 128, G4])
    din("wdih2", [2, 128, G4])
    din("bdec0", [1, G4])
    din("bdec1", [1, G4])
    din("bdec2", [1, G4])
    din("projt", [2, 128, 256])
    din("projb", [1, 256])
    din("i128", [128, 128])
    din("id8", [64, 64])
    din("ones128", [1, 128])
    din("ones4", [1, 4])
    y = nc.dram_tensor("y", [BC, dec_steps, 256], F32, kind="ExternalOutput").ap()

    with tile.TileContext(nc) as tc, ExitStack() as ctx:
        sb = ctx.enter_context(tc.tile_pool(name="sb", bufs=1))

        def load_multi(name, nch, width, tag=None):
            """dram [nch,128,width] -> sbuf tile [128, nch*width]; returns tile."""
            tl = sb.tile([128, nch * width], F32, name=f"t_{name}", tag=tag or f"t_{name}")
            for kc in range(nch):
                nc.sync.dma_start(tl[:, kc * width:(kc + 1) * width], inp[name][kc])
            return tl

        def load_flat(name, tag=None):
            shp = list(inp[name].shape)
            tl = sb.tile(shp, F32, name=f"t_{name}", tag=tag or f"t_{name}")
            nc.sync.dma_start(tl[:], inp[name][:])
            return tl

        def wch(tl, kc, n0=0, n1=G4):
            return tl[:, kc * G4 + n0: kc * G4 + n1]

        oh_t = load_multi("onehot", 2, t_steps * BC)
        i128_t = load_flat("i128")
        id8_t = load_flat("id8")
        ones128_t = load_flat("ones128")
        ones4_t = load_flat("ones4")
        projt_t = load_multi("projt", 2, 256)
        projb_t = load_flat("projb")

        # hT buffers: [128, 2*cb*4]; chunk c at col offset c*cb*4
        hT = {}
        for nm, tag in (("h0f", "hTa"), ("h0b", "hTb"), ("h1f", "hTc"),
                        ("h1b", "hTd"), ("h2b", "hTa")):
            hT[nm] = sb.tile([128, 2 * cb * 4], F32, name=f"hT_{nm}", tag=tag)
        x0_t = sb.tile([128, 8], F32, name="x0_t")

        tanh_go = [sb.tile([128, 256], F32, name=f"tgo{d}") for d in range(2)]
        cell_ab = [sb.tile([4, 512], F32, name=f"cab{d}") for d in range(2)]
        cst = [sb.tile([4, 256], F32, name=f"cst{d}") for d in range(2)]
        packed = [sb.tile([64, 128], F32, name=f"pk{d}") for d in range(2)]
        xp_sb = [[sb.tile([128, G4], F32, name=f"xps{d}_{i}") for i in range(2)]
                 for d in range(2)]

        # PSUM pools (8 banks):
        # gates 1 + tanhif 1 + T 1 + tr0 1 + tr1 1 + xpp0 1 + xpp1 1 + lg 1
        psum = ctx.enter_context(tc.tile_pool(name="psum", bufs=1, space="PSUM"))
        gates_t = psum.tile([128, 512], F32, name="gates")
        tanhif_ps = psum.tile([64, 512], F32, name="tanhif")
        T_ps = psum.tile([4, 512], F32, name="Tps")
        tr_ps = [psum.tile([128, 64], F32, name=f"tr{d}") for d in range(2)]
        xp_ps = [psum.tile([128, 512], F32, name=f"xpp{d}") for d in range(2)]
        lg_ps = psum.tile([4, 256], F32, name="lg")

        for d in range(2):
            nc.vector.memset(packed[d][:], 0.0)
        nc.vector.memset(gates_t[:], 0.0)
        nc.vector.memset(tanhif_ps[:], 0.0)
        nc.vector.memset(T_ps[:], 0.0)

        # ------------- helpers
        def ht_block(buf, w):
            return buf[:].rearrange("p (c n) -> p c n", c=2)[:, :, 4 * w:4 * w + 4]

        def ht_lhs(buf, c, w):
            o = c * cb * 4 + 4 * w
            return buf[:, o:o + 4]

        def emit_big_xp(d, slot, lhs_chunks, w_tl, nch, bias_tl):
            dst = xp_sb[d][slot]
            for half in range(2):
                n0 = half * 512
                last = nch - 1
                for kc in range(nch):
                    nc.tensor.matmul(xp_ps[d][:, :], lhs_chunks[kc],
                                     wch(w_tl, kc, n0, n0 + 512),
                                     start=(kc == 0),
                                     stop=(kc == last and bias_tl is None))
                if bias_tl is not None:
                    nc.tensor.matmul(xp_ps[d][:, :], ones128_t[:],
                                     bias_tl[:, n0:n0 + 512], start=False, stop=True)
                nc.vector.tensor_copy(dst[:, n0:n0 + 512], xp_ps[d][:, :])

        def cell_ops(d, dcol, c_state, ht_dst_ap, trd):
            nc.scalar.activation(tanhif_ps[0:64, dcol:dcol + 256],
                                 gates_t[0:64, dcol:dcol + 256], ACT.Tanh)
            nc.scalar.activation(tanh_go[d][64:128, :],
                                 gates_t[64:128, dcol:dcol + 256], ACT.Tanh)
            nc.vector.scalar_tensor_tensor(
                cell_ab[d][0:4, 256:512], tanhif_ps[32:36, dcol:dcol + 256], 1.0,
                c_state[0:4, :], OP.add, OP.mult)
            nc.vector.scalar_tensor_tensor(
                cell_ab[d][0:4, 0:256], tanhif_ps[0:4, dcol:dcol + 256], 1.0,
                tanh_go[d][64:68, :], OP.add, OP.mult)
            nc.vector.scalar_tensor_tensor(
                c_state[0:4, :], cell_ab[d][0:4, 256:512], 0.5,
                cell_ab[d][0:4, 0:256], OP.mult, OP.add)
            nc.scalar.activation(T_ps[0:4, dcol:dcol + 256], c_state[0:4, :],
                                 ACT.Tanh, scale=0.5)
            nc.vector.scalar_tensor_tensor(
                packed[d][0:4, :], tanh_go[d][96:100, 0:128], 1.0,
                T_ps[0:4, dcol:dcol + 128], OP.add, OP.mult)
            nc.vector.scalar_tensor_tensor(
                packed[d][32:36, :], tanh_go[d][96:100, 128:256], 1.0,
                T_ps[0:4, dcol + 128:dcol + 256], OP.add, OP.mult)
            nc.tensor.transpose(tr_ps[trd][:, :], packed[d][:, :], id8_t[:])
            nc.vector.tensor_copy(ht_dst_ap,
                                  tr_ps[trd][:, 0:8].rearrange("p (c b) -> p c b", c=2))

        # ------------- encoder phase
        def enc_phase(dirs, whh_tl, xp_src, out_ht):
            for d in range(len(dirs)):
                nc.vector.memset(cst[d][:], 0.0)
                w0 = 0 if dirs[d] == "f" else t_steps
                nc.vector.memset(ht_block(out_ht[d], w0), 0.0)
                blk0 = 0 if dirs[d] == "f" else nblk - 1
                xp_src(d, blk0 % 2, blk0)
            for s in range(t_steps):
                if s % 32 == 0:
                    for d in range(len(dirs)):
                        fwd = dirs[d] == "f"
                        blk = (s // 32) if fwd else (nblk - 1 - s // 32)
                        nb = blk + 1 if fwd else blk - 1
                        if 0 <= nb < nblk:
                            xp_src(d, nb % 2, nb)
                for d in range(len(dirs)):
                    fwd = dirs[d] == "f"
                    t = s if fwd else t_steps - 1 - s
                    blk = t // 32
                    tq = t % 32
                    dcol = d * 256
                    xp = xp_sb[d][blk % 2]
                    wprev = t if fwd else t + 1
                    for g in range(4):
                        nc.tensor.matmul(
                            gates_t[32 * g:32 * g + 4, dcol:dcol + 256],
                            i128_t[:, 4 * tq:4 * tq + 4],
                            xp[:, 256 * g:256 * g + 256],
                            start=True, stop=False, tile_position=(0, 32 * g))
                        for kc in range(2):
                            nc.tensor.matmul(
                                gates_t[32 * g:32 * g + 4, dcol:dcol + 256],
                                ht_lhs(out_ht[d], kc, wprev),
                                wch(whh_tl[d], kc, 256 * g, 256 * g + 256),
                                start=False, stop=(kc == 1),
                                tile_position=(0, 32 * g))
                    wdst = t + 1 if fwd else t
                    cell_ops(d, dcol, cst[d], ht_block(out_ht[d], wdst), d)

        # ---- phase A: layer 0
        penc0 = [load_multi("penc0f", 2, G4, tag="tabf"),
                 load_multi("penc0b", 2, G4, tag="tabb")]
        whh0 = [load_multi("whh0f", 2, G4, tag="whhf"),
                load_multi("whh0b", 2, G4, tag="whhb")]

        def xp_src_l0(d, slot, blk):
            lhs = [oh_t[:, kc * t_steps * BC + 128 * blk:
                         kc * t_steps * BC + 128 * blk + 128] for kc in range(2)]
            emit_big_xp(d, slot, lhs, penc0[d], 2, None)

        enc_phase(["f", "b"], whh0, xp_src_l0, [hT["h0f"], hT["h0b"]])

        # ---- phase B: layer 1
        wih1 = [load_multi("wih1f", 4, G4, tag="wihf"),
                load_multi("wih1b", 4, G4, tag="wihb")]
        whh1 = [load_multi("whh1f", 2, G4, tag="whhf"),
                load_multi("whh1b", 2, G4, tag="whhb")]
        bias1 = [load_flat("bias1f"), load_flat("bias1b")]

        def xp_lhs_cat(hf, hb, blk):
            return ([hf[:, c * cb * 4 + 4 + 128 * blk: c * cb * 4 + 4 + 128 * blk + 128]
                     for c in range(2)]
                    + [hb[:, c * cb * 4 + 128 * blk: c * cb * 4 + 128 * blk + 128]
                       for c in range(2)])

        def xp_src_l1(d, slot, blk):
            emit_big_xp(d, slot, xp_lhs_cat(hT["h0f"], hT["h0b"], blk),
                        wih1[d], 4, bias1[d][:])

        enc_phase(["f", "b"], whh1, xp_src_l1, [hT["h1f"], hT["h1b"]])

        # ---- phase C: layer 2 bwd full
        wih2b = load_multi("wih2b", 4, G4, tag="wihb")
        whh2b = [load_multi("whh2b", 2, G4, tag="whhb")]
        bias2b = load_flat("bias2b")

        def xp_src_l2b(d, slot, blk):
            emit_big_xp(d, slot, xp_lhs_cat(hT["h1f"], hT["h1b"], blk),
                        wih2b, 4, bias2b[:])

        enc_phase(["b"], whh2b, xp_src_l2b, [hT["h2b"]])

        # ---- layer 2 fwd, t=0 only
        wih2f = load_multi("wih2f", 4, G4, tag="wihf")
        bias2f = load_flat("bias2f")
        nc.vector.memset(cst[0][:], 0.0)
        lhs0 = [ht_lhs(hT["h1f"], 0, 1), ht_lhs(hT["h1f"], 1, 1),
                ht_lhs(hT["h1b"], 0, 0), ht_lhs(hT["h1b"], 1, 0)]
        for g in range(4):
            for kc in range(4):
                nc.tensor.matmul(gates_t[32 * g:32 * g + 4, 0:256], lhs0[kc],
                                 wch(wih2f, kc, 256 * g, 256 * g + 256),
                                 start=(kc == 0), stop=False,
                                 tile_position=(0, 32 * g))
            nc.tensor.matmul(gates_t[32 * g:32 * g + 4, 0:256], ones4_t[:],
                             bias2f[:, 256 * g:256 * g + 256],
                             start=False, stop=True, tile_position=(0, 32 * g))
        cell_ops(0, 0, cst[0], x0_t[:].rearrange("p (c b) -> p c b", c=2), 0)

        # ================= decoder =================
        p0 = load_multi("p0", 2, G4, tag="tabf")
        wx0 = load_multi("wx0", 4, G4, tag="wihf")
        wdhh = [load_multi("wdhh0", 2, G4, tag="whhf"),
                load_multi("wdhh1", 2, G4, tag="whhb"),
                load_multi("wdhh2", 2, G4, tag="tabb")]
        wdih = {1: load_multi("wdih1", 2, G4, tag="wdih1"),
                2: load_multi("wdih2", 2, G4, tag="wdih2")}
        bdec = {0: load_flat("bdec0"), 1: load_flat("bdec1"), 2: load_flat("bdec2")}
        hts = [sb.tile([128, 16], F32, name=f"hts{l}") for l in range(3)]
        maskT = sb.tile([128, 16], F32, name="maskT")
        m_t = sb.tile([4, 1], F32, name="m_t")
        lg_sb = sb.tile([4, 512], F32, name="lg_sb")
        cstd = [sb.tile([4, 256], F32, name=f"cstd{l}") for l in range(3)]
        for l in range(3):
            nc.vector.memset(cstd[l][:], 0.0)

        for t in range(dec_steps):
            # layer 0
            if t == 0:
                x0c = [x0_t[:, 0:4], x0_t[:, 4:8],
                       ht_lhs(hT["h2b"], 0, 0), ht_lhs(hT["h2b"], 1, 0)]
                for g in range(4):
                    for kc in range(4):
                        nc.tensor.matmul(gates_t[32 * g:32 * g + 4, 0:256], x0c[kc],
                                         wch(wx0, kc, 256 * g, 256 * g + 256),
                                         start=(kc == 0), stop=False,
                                         tile_position=(0, 32 * g))
                    nc.tensor.matmul(gates_t[32 * g:32 * g + 4, 0:256], ones4_t[:],
                                     bdec[0][:, 256 * g:256 * g + 256],
                                     start=False, stop=True, tile_position=(0, 32 * g))
            else:
                msl = (t % 2) * 8
                hsl = ((t + 1) % 2) * 8
                for g in range(4):
                    for kc in range(2):
                        nc.tensor.matmul(gates_t[32 * g:32 * g + 4, 0:256],
                                         maskT[:, msl + 4 * kc:msl + 4 * kc + 4],
                                         wch(p0, kc, 256 * g, 256 * g + 256),
                                         start=(kc == 0), stop=False,
                                         tile_position=(0, 32 * g))
                    for kc in range(2):
                        nc.tensor.matmul(gates_t[32 * g:32 * g + 4, 0:256],
                                         hts[0][:, hsl + 4 * kc:hsl + 4 * kc + 4],
                                         wch(wdhh[0], kc, 256 * g, 256 * g + 256),
                                         start=False, stop=(kc == 1),
                                         tile_position=(0, 32 * g))
            sl = (t % 2) * 8
            cell_ops(0, 0, cstd[0],
                     hts[0][:, sl:sl + 8].rearrange("p (c b) -> p c b", c=2), 0)
            # layers 1, 2
            for l in (1, 2):
                hsl = ((t + 1) % 2) * 8
                for g in range(4):
                    for kc in range(2):
                        nc.tensor.matmul(gates_t[32 * g:32 * g + 4, 0:256],
                                         hts[l - 1][:, sl + 4 * kc:sl + 4 * kc + 4],
                                         wch(wdih[l], kc, 256 * g, 256 * g + 256),
                                         start=(kc == 0), stop=False,
                                         tile_position=(0, 32 * g))
                    if t > 0:
                        for kc in range(2):
                            nc.tensor.matmul(gates_t[32 * g:32 * g + 4, 0:256],
                                             hts[l][:, hsl + 4 * kc:hsl + 4 * kc + 4],
                                             wch(wdhh[l], kc, 256 * g, 256 * g + 256),
                                             start=False, stop=False,
                                             tile_position=(0, 32 * g))
                    nc.tensor.matmul(gates_t[32 * g:32 * g + 4, 0:256], ones4_t[:],
                                     bdec[l][:, 256 * g:256 * g + 256],
                                     start=False, stop=True,
                                     tile_position=(0, 32 * g))
                cell_ops(0, 0, cstd[l],
                         hts[l][:, sl:sl + 8].rearrange("p (c b) -> p c b", c=2), 0)
            # logits
            for kc in range(2):
                nc.tensor.matmul(lg_ps[:, :], hts[2][:, sl + 4 * kc:sl + 4 * kc + 4],
                                 projt_t[:, kc * 256:(kc + 1) * 256],
                                 start=(kc == 0), stop=False)
            nc.tensor.matmul(lg_ps[:, :], ones4_t[:], projb_t[:],
                             start=False, stop=True)
            lsl = (t % 2) * 256
            nc.vector.tensor_copy(lg_sb[:, lsl:lsl + 256], lg_ps[:, :])
            nc.sync.dma_start(y[:, t, :], lg_sb[:, lsl:lsl + 256])
            # argmax mask for next step
            if t + 1 < dec_steps:
                nc.vector.tensor_reduce(m_t[:], lg_ps[:, :], axis=AX.X, op=OP.max)
                nc.vector.tensor_scalar(packed[1][0:4, :], lg_ps[0:4, 0:128],
                                        m_t[:], None, OP.is_ge)
                nc.vector.tensor_scalar(packed[1][32:36, :], lg_ps[0:4, 128:256],
                                        m_t[:], None, OP.is_ge)
                nc.tensor.transpose(tr_ps[1][:, :], packed[1][:, :], id8_t[:])
                msl = ((t + 1) % 2) * 8
                nc.vector.tensor_copy(maskT[:, msl:msl + 8], tr_ps[1][:, 0:8])

    nc.compile()
    return nc, inp


# ---------------------------------------------------------------- entry point
def kernel(text_bytes, max_nefs_len, embedding, enc_params, dec_params,
           proj_w, proj_b):
    from concourse.bass_utils import run_bass_kernel_spmd

    text = np.asarray(text_bytes)
    tabs = prep_tables(embedding, enc_params, dec_params, proj_w, proj_b)

    if "full" not in _PROGRAM_CACHE:
        _PROGRAM_CACHE["full"] = build_program()
    nc, _ = _PROGRAM_CACHE["full"]

    in_maps = []
    for c in range(NCORES):
        m = dict(tabs)
        m["onehot"] = onehot_core(text[c * BC:(c + 1) * BC])
        in_maps.append(m)

    trace = bool(int(os.environ.get("K_TRACE", "0")))
    res = run_bass_kernel_spmd(nc, in_maps, list(range(NCORES)), trace=trace)
    out = np.concatenate([res.results[c]["y"] for c in range(NCORES)], axis=0)
    kernel.last_results = res
    return out.astype(np.float32)
